# revision 49
# baseline (speedup 1.0000x reference)
"""Multi-head attention (S=2048, B=2, E=1024, H=16) on 8 Trainium2 cores.

Sharding: data-parallel over batch (4 cores per batch element) x tensor-parallel
over heads (4 heads per core), Megatron-style: Wq/Wk/Wv column-sharded,
Wo row-sharded, per-core partial outputs summed (+bo) on host.

V2 (vs V1 baseline at 295.7us):
- q/k/outU/outT stored PAIR-STACKED [128, T] (head 2p rows 0:64, head 2p+1 rows
  64:128). Score matmuls for the two heads of a pair run CONCURRENTLY as PE
  row-tiles T0/T8 (tile_position (0,0)/(64,0), auto-derived from base
  partitions): one 512-cycle streaming window computes both heads' scores.
- PE warmup: ~40 dummy matmuls on a memset tile at kernel start flip the HAM
  clock gate to 8/8 before real matmuls arrive (baseline ran its first ~39us
  at 1.2GHz).
- Constants built with DVE memset (no DMA); biases in 3 small DMAs issued
  after the first xk tile.
- DMA order: wk, xk (k-tiles), wq, xq-n0-block, wv, xv, xq-n1..3, wo. xq is
  staged in sq-block-major dram layout so the first scores only need
  wk+xk+wq+xq_n0 (6MB) instead of 9MB.
- exp run-ahead: 16 ex buffers decouple ACT (the inner-loop pacer, 1147ns per
  [128,1024] exp) from lagging PV matmuls (vbuf lands late); V-projection and
  late Q-projection matmuls fill PE slack inside the ACT-paced loops via the
  readiness-driven Tile scheduler.
- Normalize: one K=2 broadcast matmul per pair (selector lhsT) broadcasts both
  heads' 1/denom rows across partitions; outT = outU * pb multiplies straight
  from PSUM. Softmax denominators still ride row 64 of the PV matmuls (M=65).
- o-projection evicted as [128,1024] tiles (2 m-blocks per PSUM tile/eviction/
  DMA).
PSUM: psc 3x[128,1024] (6 banks: score/exp double-buffer + scratch for v-proj,
o-proj, bcast, projections) + po 2x[128,512] (PV accumulators; also the last 2
projection groups). K/Q projections run k-outer across 8 concurrent groups.
"""

import numpy as np
import ml_dtypes
from contextlib import ExitStack

import concourse.bass as bass
import concourse.mybir as mybir
from concourse import bacc
import concourse.tile as tile
from concourse.bass_utils import run_bass_kernel_spmd

S, B, E, H, HD = 2048, 2, 1024, 16, 64
P = 128
NCORES = 8
CORES_PER_BATCH = 4
HEADS_PER_CORE = H // CORES_PER_BATCH      # 4
LOCAL_E = HEADS_PER_CORE * HD              # 256
VW = HD + 1                                # 65: [v_h | ones]
T = S                                      # tokens per core (one batch elem)
KT = E // P                                # 8 contraction tiles for projections
NPAIR = HEADS_PER_CORE // 2                # 2 head pairs
SQ_BLK = 512
NSQ = T // SQ_BLK                          # 4
NSK = T // P                               # 16
F32 = mybir.dt.float32
F32R = mybir.dt.float32r
BF16 = mybir.dt.bfloat16
NPBF16 = ml_dtypes.bfloat16
EXPF = mybir.ActivationFunctionType.Exp
ADD = mybir.AluOpType.add
MULT = mybir.AluOpType.mult

N_WARMUP = 40


def _build_program():
    nc = bacc.Bacc("TRN2")

    xk = nc.dram_tensor("xk", [E, T], BF16, kind="ExternalInput")
    # xq staged sq-block-major: block n holds xT[:, n*512:(n+1)*512]
    xq = nc.dram_tensor("xq", [NSQ * E, SQ_BLK], BF16, kind="ExternalInput")
    xv = nc.dram_tensor("xv", [E, T], BF16, kind="ExternalInput")
    wqt = nc.dram_tensor("wqt", [E, LOCAL_E], BF16, kind="ExternalInput")
    wkt = nc.dram_tensor("wkt", [E, LOCAL_E], BF16, kind="ExternalInput")
    wvt = nc.dram_tensor("wvt", [E, LOCAL_E], BF16, kind="ExternalInput")
    wot = nc.dram_tensor("wot", [LOCAL_E, E], BF16, kind="ExternalInput")
    # [bq_pair0 | bq_pair1 | bk_pair0 | bk_pair1], 128 rows each
    bqk = nc.dram_tensor("bqk", [2 * LOCAL_E], F32, kind="ExternalInput")
    # per head: [bv_head (64), 1.0] -> 65 columns
    bvb = nc.dram_tensor("bvb", [HEADS_PER_CORE * VW], F32R,
                         kind="ExternalInput")
    outp = nc.dram_tensor("outp", [E, T], F32, kind="ExternalOutput")

    with ExitStack() as ctx:
        ctx.enter_context(nc.allow_low_precision(reason="bf16 matmul pipeline"))
        tc = ctx.enter_context(tile.TileContext(nc))
        cpool = ctx.enter_context(tc.tile_pool(name="cpool", bufs=1))
        spool = ctx.enter_context(tc.tile_pool(name="spool", bufs=2))
        xpool = ctx.enter_context(tc.tile_pool(name="xpool", bufs=6))
        xqpool = ctx.enter_context(tc.tile_pool(name="xqpool", bufs=4))
        wpool = ctx.enter_context(tc.tile_pool(name="wpool", bufs=1))
        qkpool = ctx.enter_context(tc.tile_pool(name="qkpool", bufs=4))
        vpool = ctx.enter_context(tc.tile_pool(name="vpool", bufs=1))
        opool = ctx.enter_context(tc.tile_pool(name="opool", bufs=4))
        epool = ctx.enter_context(tc.tile_pool(name="epool", bufs=16))
        fpool = ctx.enter_context(tc.tile_pool(name="fpool", bufs=2))
        # PSUM (8 banks): psc 2x[128,1024] (score/exp double-buffer, 4 banks)
        # + pox 1x[128,1024] (projection/normalize/o-proj scratch, 2 banks)
        # + po 2x[128,512] (PV accumulators, 2 banks)
        psc = ctx.enter_context(tc.tile_pool(name="psc", bufs=2, space="PSUM"))
        pox = ctx.enter_context(tc.tile_pool(name="pox", bufs=1, space="PSUM"))
        po = ctx.enter_context(tc.tile_pool(name="po", bufs=2, space="PSUM"))

        # ---- memset constants (no DMA) ---------------------------------
        wtile = cpool.tile([P, P], BF16, tag="warm")
        nc.vector.memset(wtile[:], 0.0)
        onesf = cpool.tile([1, P], F32, tag="onesf")
        nc.vector.memset(onesf[:], 1.0)
        onesr = cpool.tile([1, P], F32R, tag="onesr")
        nc.vector.tensor_copy(onesr[:], onesf[:])

        # ---- PE warmup: flip HAM to 8/8 during the DMA lead-in ---------
        wps = pox.tile([P, 2 * SQ_BLK], F32, tag="px", name="warmps")
        for i in range(N_WARMUP):
            nc.tensor.matmul(wps[:, 0:P], wtile[:], wtile[:],
                             start=True, stop=True)

        # ---- DMAs in need-order (few, large) ---------------------------
        # x tiles hold 2 k-tiles each (1MB transfers)
        wk_sb = wpool.tile([P, KT, LOCAL_E], BF16, tag="wk")
        nc.sync.dma_start(wk_sb[:], wkt.rearrange("(k p) n -> p k n", p=P))
        xkt2 = [xpool.tile([P, 2, T], BF16, tag="x", name=f"xk{k}")
                for k in range(KT // 2)]
        nc.sync.dma_start(
            xkt2[0][:], xk[0:2 * P, :].rearrange("(g p) t -> p g t", p=P))
        bqk_sb = cpool.tile([P, 4], F32, tag="bqk")
        nc.sync.dma_start(bqk_sb[:], bqk.rearrange("(c p) -> p c", p=P))
        bvb_sb = cpool.tile([1, HEADS_PER_CORE * VW], F32R, tag="bvbrow")
        nc.sync.dma_start(bvb_sb[:], bvb[None, :])
        for k in range(1, KT // 2):
            nc.sync.dma_start(
                xkt2[k][:],
                xk[2 * k * P:2 * (k + 1) * P, :].rearrange(
                    "(g p) t -> p g t", p=P))
        wq_sb = wpool.tile([P, KT, LOCAL_E], BF16, tag="wq")
        nc.sync.dma_start(wq_sb[:], wqt.rearrange("(k p) n -> p k n", p=P))
        xqn = [xqpool.tile([P, KT, SQ_BLK], BF16, tag="xq", name=f"xq{n}")
               for n in range(NSQ)]
        nc.sync.dma_start(
            xqn[0][:], xq[0:E, :].rearrange("(k p) n -> p k n", p=P))
        wv_sb = wpool.tile([P, KT, LOCAL_E], BF16, tag="wv")
        nc.sync.dma_start(wv_sb[:], wvt.rearrange("(k p) n -> p k n", p=P))
        xvt2 = [xpool.tile([P, 2, T], BF16, tag="x", name=f"xv{k}")
                for k in range(KT // 2)]
        for k in range(KT // 2):
            nc.sync.dma_start(
                xvt2[k][:],
                xv[2 * k * P:2 * (k + 1) * P, :].rearrange(
                    "(g p) t -> p g t", p=P))
        for n in range(1, NSQ):
            nc.sync.dma_start(
                xqn[n][:], xq[n * E:(n + 1) * E, :].rearrange(
                    "(k p) n -> p k n", p=P))
        wo_sb = wpool.tile([P, LOCAL_E // P, E], BF16, tag="wo")
        nc.sync.dma_start(wo_sb[:], wot.rearrange("(k p) n -> p k n", p=P))

        def xk_tile(k):
            return xkt2[k // 2][:, k % 2, :]

        def xv_tile(k):
            return xvt2[k // 2][:, k % 2, :]

        # ---- bvb broadcast across partitions (K=1 matmul) --------------
        bvb_ps = pox.tile([P, 2 * SQ_BLK], F32, tag="px", name="bvbps")
        nc.tensor.matmul(bvb_ps[:, 0:HEADS_PER_CORE * VW],
                         onesr[0:1, :], bvb_sb[0:1, :],
                         start=True, stop=True)
        bvb_bc = cpool.tile([P, HEADS_PER_CORE * VW], F32, tag="bvbbc")
        nc.vector.tensor_copy(bvb_bc[:], bvb_ps[:, 0:HEADS_PER_CORE * VW])

        # ---- persistent activations ------------------------------------
        # pair-stacked: rows 0:64 = head 2p, rows 64:128 = head 2p+1
        kp = [qkpool.tile([P, T], BF16, tag="qk", name=f"kp{i}")
              for i in range(NPAIR)]
        qp = [qkpool.tile([P, T], BF16, tag="qk", name=f"qp{i}")
              for i in range(NPAIR)]
        # v buffer: per sk-tile, per head: [v_h (64 cols) | ones (1 col)]
        vbuf = vpool.tile([P, NSK, HEADS_PER_CORE * VW], BF16, tag="v")
        outU = [opool.tile([P, T], BF16, tag="o", name=f"outU{i}")
                for i in range(NPAIR)]
        outT = [opool.tile([P, T], BF16, tag="o", name=f"outT{i}")
                for i in range(NPAIR)]

        # ---- K projection: k-outer over 8 concurrent PSUM groups ------
        def kq_proj_full(w_sb, xt, bias_col, dst):
            gt = [psc.tile([P, 2 * SQ_BLK], F32, tag="sc", name=f"pg{i}")
                  for i in range(2)]
            gx = pox.tile([P, 2 * SQ_BLK], F32, tag="px", name="pgx")
            gp = [po.tile([P, SQ_BLK], F32, tag="po", name=f"pgp{i}")
                  for i in range(2)]
            groups = [gt[0][:, 0:SQ_BLK], gt[0][:, SQ_BLK:2 * SQ_BLK],
                      gt[1][:, 0:SQ_BLK], gt[1][:, SQ_BLK:2 * SQ_BLK],
                      gx[:, 0:SQ_BLK], gx[:, SQ_BLK:2 * SQ_BLK],
                      gp[0][:, :], gp[1][:, :]]
            for k in range(KT):
                for pr in range(NPAIR):
                    for n in range(NSQ):
                        nc.tensor.matmul(
                            groups[pr * NSQ + n],
                            w_sb[:, k, pr * P:(pr + 1) * P],
                            xt(k)[:, n * SQ_BLK:(n + 1) * SQ_BLK],
                            start=(k == 0), stop=(k == KT - 1))
            for pr in range(NPAIR):
                for n in range(NSQ):
                    nc.vector.tensor_scalar_add(
                        dst[pr][:, n * SQ_BLK:(n + 1) * SQ_BLK],
                        groups[pr * NSQ + n],
                        bqk_sb[:, bias_col + pr:bias_col + pr + 1])

        kq_proj_full(wk_sb, xk_tile, 2, kp)

        # ---- Q projection, one sq-block at a time ----------------------
        def q_proj_block(n):
            ps = pox.tile([P, 2 * SQ_BLK], F32, tag="px", name=f"qpg{n}")
            for pr in range(NPAIR):
                for k in range(KT):
                    nc.tensor.matmul(
                        ps[:, pr * SQ_BLK:(pr + 1) * SQ_BLK],
                        wq_sb[:, k, pr * P:(pr + 1) * P],
                        xqn[n][:, k, :],
                        start=(k == 0), stop=(k == KT - 1))
            for pr in range(NPAIR):
                nc.vector.tensor_scalar_add(
                    qp[pr][:, n * SQ_BLK:(n + 1) * SQ_BLK],
                    ps[:, pr * SQ_BLK:(pr + 1) * SQ_BLK],
                    bqk_sb[:, pr:pr + 1])

        q_proj_block(0)

        # ---- V projection: all 16 token-tiles share ONE pox tile,
        # alternating its two banks (eviction of tile t overlaps the
        # matmuls of tile t+1 without pool-slot serialization) ----------
        vps = pox.tile([P, 2 * SQ_BLK], F32, tag="px", name="vps")

        def v_proj_tile(tt):
            ps = vps[:, (tt % 2) * SQ_BLK:(tt % 2) * SQ_BLK + LOCAL_E]
            for k in range(KT):
                nc.tensor.matmul(
                    ps,
                    xv_tile(k)[:, tt * P:(tt + 1) * P],
                    wv_sb[:, k, :],
                    start=(k == 0), stop=(k == KT - 1))
            nc.vector.tensor_tensor(
                vbuf.rearrange("p s (h c) -> p s h c", c=VW)[:, tt, :, 0:HD],
                ps.rearrange("p (h c) -> p h c", c=HD),
                bvb_bc.rearrange("p (h c) -> p h c", c=VW)[:, :, 0:HD],
                ADD)
            nc.vector.tensor_copy(
                vbuf.rearrange("p s (h c) -> p s h c", c=VW)
                [:, tt, :, HD:HD + 1],
                bvb_bc.rearrange("p (h c) -> p h c", c=VW)[:, :, HD:HD + 1])

        # V projection and late Q blocks: emitted pre-attention (correct
        # program order for the dependency tracker) but priority-DEMOTED so
        # the scheduler slots them into PE slack of the ACT-paced attention
        # loops instead of running them ahead of the first scores.
        # ~5 instructions per attention sk-iteration.
        # NOTE: pox slot reuse is FIFO in call order, so demotion offsets
        # must keep pox users' priorities monotone in emission order.
        for tt in range(NSK):
            with tc.high_priority(offset=-(3 + 5 * tt)):
                v_proj_tile(tt)
        for n in range(1, NSQ):
            with tc.high_priority(offset=-(131 + 30 * (n - 1))):
                q_proj_block(n)

        def make_normalize_a(sq, pr, po_t, rcr):
            """DVE-only part: denominators + outU eviction (releases po)."""
            sqs = slice(sq * SQ_BLK, (sq + 1) * SQ_BLK)

            def emit():
                dn = spool.tile([1, 2 * SQ_BLK], F32, tag="dn",
                                name=f"dn{pr}_{sq}")
                for X in range(2):
                    nc.vector.tensor_copy(
                        dn[0:1, X * SQ_BLK:(X + 1) * SQ_BLK],
                        po_t[X][HD:VW, :])
                rc = spool.tile([1, 2 * SQ_BLK], F32, tag="rc",
                                name=f"rc{pr}_{sq}")
                nc.vector.reciprocal_approx_fast(rc[:], dn[:])
                nc.vector.tensor_copy(rcr[:], rc[:])
                for X in range(2):
                    nc.vector.tensor_copy(
                        outU[pr][X * HD:(X + 1) * HD, sqs], po_t[X][0:HD, :])
            return emit

        def make_normalize_b(sq, pr, rcr):
            """PE broadcast + multiply (outT)."""
            sqs = slice(sq * SQ_BLK, (sq + 1) * SQ_BLK)

            def emit():
                pb = pox.tile([P, 2 * SQ_BLK], F32, tag="px",
                              name=f"pb{pr}_{sq}")
                for X in range(2):
                    nc.tensor.matmul(pb[:, X * SQ_BLK:(X + 1) * SQ_BLK],
                                     onesr[0:1, :],
                                     rcr[0:1, X * SQ_BLK:(X + 1) * SQ_BLK],
                                     start=True, stop=True)
                    nc.vector.tensor_tensor(
                        outT[pr][X * HD:(X + 1) * HD, sqs],
                        outU[pr][X * HD:(X + 1) * HD, sqs],
                        pb[X * HD:(X + 1) * HD, X * SQ_BLK:(X + 1) * SQ_BLK],
                        MULT)
            return emit

        def make_oproj(sq, mp, pool):
            sqs = slice(sq * SQ_BLK, (sq + 1) * SQ_BLK)

            def emit():
                pst = pool.tile([P, 2 * SQ_BLK], F32,
                                tag="sc" if pool is psc else "px",
                                name=f"op{sq}_{mp}")
                for half in range(2):
                    m = 2 * mp + half
                    for kb in range(NPAIR):
                        nc.tensor.matmul(
                            pst[:, half * SQ_BLK:(half + 1) * SQ_BLK],
                            wo_sb[:, kb, m * P:(m + 1) * P],
                            outT[kb][:, sqs],
                            start=(kb == 0), stop=(kb == NPAIR - 1))
                fin = fpool.tile([P, 2 * SQ_BLK], F32, tag="f")
                nc.vector.tensor_copy(fin[:], pst[:])
                nc.sync.dma_start(
                    outp[mp * 2 * P:(mp + 1) * 2 * P, sqs].rearrange(
                        "(g p) n -> p g n", p=P),
                    fin.rearrange("p (g n) -> p g n", g=2))
            return emit

        # ---- attention: normalize/o-proj deferred into the next pair's
        # loop; fill units popped every other iteration ------------------
        pending = []
        for sq in range(NSQ):
            sqs = slice(sq * SQ_BLK, (sq + 1) * SQ_BLK)
            for pr in range(NPAIR):
                po_t = [po.tile([P, SQ_BLK], F32, tag="po",
                                name=f"po{pr}_{sq}_{i}") for i in range(2)]
                for sk in range(NSK):
                    sks = slice(sk * P, (sk + 1) * P)
                    ps = psc.tile([P, 2 * SQ_BLK], F32, tag="sc")
                    # both heads' scoresT concurrently (PE row-tiles T0/T8)
                    nc.tensor.matmul(ps[:, 0:SQ_BLK],
                                     kp[pr][0:HD, sks], qp[pr][0:HD, sqs],
                                     start=True, stop=True)
                    nc.tensor.matmul(ps[:, SQ_BLK:2 * SQ_BLK],
                                     kp[pr][HD:P, sks], qp[pr][HD:P, sqs],
                                     start=True, stop=True)
                    ex = epool.tile([P, 2 * SQ_BLK], BF16, tag="e")
                    nc.scalar.activation(ex[:], ps[:], EXPF)
                    for X in range(2):
                        h = 2 * pr + X
                        nc.tensor.matmul(
                            po_t[X][0:VW, :],
                            vbuf[:, sk, h * VW:(h + 1) * VW],
                            ex[:, X * SQ_BLK:(X + 1) * SQ_BLK],
                            start=(sk == 0), stop=(sk == NSK - 1))
                    # deferred consumers from the previous pair / sq
                    if sk == 1 and pending:
                        pending.pop(0)()
                    elif sk >= 4 and sk % 2 == 0 and pending:
                        pending.pop(0)()
                rcr = spool.tile([1, 2 * SQ_BLK], F32R, tag="rcr",
                                 name=f"rcr{pr}_{sq}")
                pending.append(make_normalize_a(sq, pr, po_t, rcr))
                pending.append(make_normalize_b(sq, pr, rcr))
            for mp in range(E // (2 * P)):
                # final sq's o-proj alternates pools so the tail pipelines
                pool = psc if (sq == NSQ - 1 and mp % 2 == 0) else pox
                pending.append(make_oproj(sq, mp, pool))
        for p in pending:
            p()

    nc.compile()
    return nc


_NC = None


def _get_nc():
    global _NC
    if _NC is None:
        _NC = _build_program()
    return _NC


def _bf(a):
    return np.ascontiguousarray(a.astype(NPBF16))


def _make_in_maps(inputs):
    q = np.asarray(inputs["query"], np.float32)
    k = np.asarray(inputs["key"], np.float32)
    v = np.asarray(inputs["value"], np.float32)
    Wq = np.asarray(inputs["Wq"], np.float32)
    Wk = np.asarray(inputs["Wk"], np.float32)
    Wv = np.asarray(inputs["Wv"], np.float32)
    Wo = np.asarray(inputs["Wo"], np.float32)
    bq = np.asarray(inputs["bq"], np.float32)
    bk = np.asarray(inputs["bk"], np.float32)
    bv = np.asarray(inputs["bv"], np.float32)
    scale = np.float32(HD ** -0.5)

    in_maps = []
    for c in range(NCORES):
        b = c // CORES_PER_BATCH
        h0 = (c % CORES_PER_BATCH) * HEADS_PER_CORE
        hsl = slice(h0 * HD, (h0 + HEADS_PER_CORE) * HD)
        bvh = bv[hsl].reshape(HEADS_PER_CORE, HD)
        bvb = np.concatenate(
            [bvh, np.ones((HEADS_PER_CORE, 1), np.float32)], axis=1).ravel()
        xqT = q[:, b, :].T                                  # [E, T]
        xq_blocks = np.concatenate(
            [xqT[:, n * SQ_BLK:(n + 1) * SQ_BLK] for n in range(NSQ)], axis=0)
        in_maps.append({
            "xq": _bf(xq_blocks),
            "xk": _bf(k[:, b, :].T),
            "xv": _bf(v[:, b, :].T),
            "wqt": _bf((Wq[hsl, :] * scale).T),
            "wkt": _bf(Wk[hsl, :].T),
            "wvt": _bf(Wv[hsl, :].T),
            "wot": _bf(Wo[:, hsl].T),
            "bqk": np.ascontiguousarray(
                np.concatenate([bq[hsl] * scale, bk[hsl]]).astype(np.float32)),
            "bvb": np.ascontiguousarray(bvb.astype(np.float32)),
        })
    return in_maps


def run_sharded(inputs, trace=False):
    """Returns (full_output [S,B,E] f32, BassKernelResults)."""
    nc = _get_nc()
    in_maps = _make_in_maps(inputs)
    res = run_bass_kernel_spmd(nc, in_maps, core_ids=list(range(NCORES)),
                               trace=trace)
    bo = np.asarray(inputs["bo"], np.float32)
    final = np.zeros((S, B, E), np.float32)
    for c in range(NCORES):
        b = c // CORES_PER_BATCH
        final[:, b, :] += res.results[c]["outp"].T
    final += bo
    return final, res


def kernel(**inputs):
    out, _ = run_sharded(inputs, trace=False)
    return out


# revision 50
# speedup vs baseline: 1.1987x; 1.1987x over previous
"""Multi-head attention (S=2048, B=2, E=1024, H=16) on 8 Trainium2 cores.

Sharding: data-parallel over batch (4 cores per batch element) x tensor-parallel
over heads (4 heads per core), Megatron-style: Wq/Wk/Wv column-sharded,
Wo row-sharded, per-core partial outputs summed (+bo) on host.

V2 (vs V1 baseline at 295.7us):
- q/k/outU/outT stored PAIR-STACKED [128, T] (head 2p rows 0:64, head 2p+1 rows
  64:128). Score matmuls for the two heads of a pair run CONCURRENTLY as PE
  row-tiles T0/T8 (tile_position (0,0)/(64,0), auto-derived from base
  partitions): one 512-cycle streaming window computes both heads' scores.
- PE warmup: ~40 dummy matmuls on a memset tile at kernel start flip the HAM
  clock gate to 8/8 before real matmuls arrive (baseline ran its first ~39us
  at 1.2GHz).
- Constants built with DVE memset (no DMA); biases in 3 small DMAs issued
  after the first xk tile.
- DMA order: wk, xk (k-tiles), wq, xq-n0-block, wv, xv, xq-n1..3, wo. xq is
  staged in sq-block-major dram layout so the first scores only need
  wk+xk+wq+xq_n0 (6MB) instead of 9MB.
- exp run-ahead: 16 ex buffers decouple ACT (the inner-loop pacer, 1147ns per
  [128,1024] exp) from lagging PV matmuls (vbuf lands late); V-projection and
  late Q-projection matmuls fill PE slack inside the ACT-paced loops via the
  readiness-driven Tile scheduler.
- Normalize: one K=2 broadcast matmul per pair (selector lhsT) broadcasts both
  heads' 1/denom rows across partitions; outT = outU * pb multiplies straight
  from PSUM. Softmax denominators still ride row 64 of the PV matmuls (M=65).
- o-projection evicted as [128,1024] tiles (2 m-blocks per PSUM tile/eviction/
  DMA).
PSUM: psc 3x[128,1024] (6 banks: score/exp double-buffer + scratch for v-proj,
o-proj, bcast, projections) + po 2x[128,512] (PV accumulators; also the last 2
projection groups). K/Q projections run k-outer across 8 concurrent groups.
"""

import numpy as np
import ml_dtypes
from contextlib import ExitStack

import concourse.bass as bass
import concourse.mybir as mybir
from concourse import bacc
import concourse.tile as tile
from concourse.bass_utils import run_bass_kernel_spmd

S, B, E, H, HD = 2048, 2, 1024, 16, 64
P = 128
NCORES = 8
CORES_PER_BATCH = 4
HEADS_PER_CORE = H // CORES_PER_BATCH      # 4
LOCAL_E = HEADS_PER_CORE * HD              # 256
VW = HD + 1                                # 65: [v_h | ones]
T = S                                      # tokens per core (one batch elem)
KT = E // P                                # 8 contraction tiles for projections
NPAIR = HEADS_PER_CORE // 2                # 2 head pairs
SQ_BLK = 512
NSQ = T // SQ_BLK                          # 4
NSK = T // P                               # 16
F32 = mybir.dt.float32
F32R = mybir.dt.float32r
BF16 = mybir.dt.bfloat16
NPBF16 = ml_dtypes.bfloat16
EXPF = mybir.ActivationFunctionType.Exp
ADD = mybir.AluOpType.add
MULT = mybir.AluOpType.mult

N_WARMUP = 40


def _build_program():
    nc = bacc.Bacc("TRN2")

    xk = nc.dram_tensor("xk", [E, T], BF16, kind="ExternalInput")
    # xq staged sq-block-major: block n holds xT[:, n*512:(n+1)*512]
    xq = nc.dram_tensor("xq", [NSQ * E, SQ_BLK], BF16, kind="ExternalInput")
    xv = nc.dram_tensor("xv", [E, T], BF16, kind="ExternalInput")
    wqt = nc.dram_tensor("wqt", [E, LOCAL_E], BF16, kind="ExternalInput")
    wkt = nc.dram_tensor("wkt", [E, LOCAL_E], BF16, kind="ExternalInput")
    wvt = nc.dram_tensor("wvt", [E, LOCAL_E], BF16, kind="ExternalInput")
    wot = nc.dram_tensor("wot", [LOCAL_E, E], BF16, kind="ExternalInput")
    # [bq_pair0 | bq_pair1 | bk_pair0 | bk_pair1], 128 rows each
    bqk = nc.dram_tensor("bqk", [2 * LOCAL_E], F32, kind="ExternalInput")
    # per head: [bv_head (64), 1.0] -> 65 columns
    bvb = nc.dram_tensor("bvb", [HEADS_PER_CORE * VW], F32R,
                         kind="ExternalInput")
    outp = nc.dram_tensor("outp", [E, T], F32, kind="ExternalOutput")

    with ExitStack() as ctx:
        ctx.enter_context(nc.allow_low_precision(reason="bf16 matmul pipeline"))
        tc = ctx.enter_context(tile.TileContext(nc))
        cpool = ctx.enter_context(tc.tile_pool(name="cpool", bufs=1))
        spool = ctx.enter_context(tc.tile_pool(name="spool", bufs=2))
        xpool = ctx.enter_context(tc.tile_pool(name="xpool", bufs=6))
        xqpool = ctx.enter_context(tc.tile_pool(name="xqpool", bufs=4))
        wpool = ctx.enter_context(tc.tile_pool(name="wpool", bufs=1))
        qkpool = ctx.enter_context(tc.tile_pool(name="qkpool", bufs=4))
        vpool = ctx.enter_context(tc.tile_pool(name="vpool", bufs=1))
        opool = ctx.enter_context(tc.tile_pool(name="opool", bufs=4))
        epool = ctx.enter_context(tc.tile_pool(name="epool", bufs=16))
        fpool = ctx.enter_context(tc.tile_pool(name="fpool", bufs=2))
        # PSUM (8 banks): psc 2x[128,1024] (score/exp double-buffer, 4 banks)
        # + pox 1x[128,1024] (projection/normalize/o-proj scratch, 2 banks)
        # + po 2x[128,512] (PV accumulators, 2 banks)
        psc = ctx.enter_context(tc.tile_pool(name="psc", bufs=2, space="PSUM"))
        pox = ctx.enter_context(tc.tile_pool(name="pox", bufs=1, space="PSUM"))
        po = ctx.enter_context(tc.tile_pool(name="po", bufs=2, space="PSUM"))

        # ---- memset constants (no DMA) ---------------------------------
        wtile = cpool.tile([P, P], BF16, tag="warm")
        nc.vector.memset(wtile[:], 0.0)
        onesf = cpool.tile([1, P], F32, tag="onesf")
        nc.vector.memset(onesf[:], 1.0)
        onesr = cpool.tile([1, P], F32R, tag="onesr")
        nc.vector.tensor_copy(onesr[:], onesf[:])

        # ---- PE warmup: flip HAM to 8/8 during the DMA lead-in ---------
        wps = pox.tile([P, 2 * SQ_BLK], F32, tag="px", name="warmps")
        for i in range(N_WARMUP):
            nc.tensor.matmul(wps[:, 0:P], wtile[:], wtile[:],
                             start=True, stop=True)

        # ---- DMAs in need-order (few, large) ---------------------------
        # x tiles hold 2 k-tiles each (1MB transfers)
        wk_sb = wpool.tile([P, KT, LOCAL_E], BF16, tag="wk")
        nc.sync.dma_start(wk_sb[:], wkt.rearrange("(k p) n -> p k n", p=P))
        xkt2 = [xpool.tile([P, 2, T], BF16, tag="x", name=f"xk{k}")
                for k in range(KT // 2)]
        nc.sync.dma_start(
            xkt2[0][:], xk[0:2 * P, :].rearrange("(g p) t -> p g t", p=P))
        bqk_sb = cpool.tile([P, 4], F32, tag="bqk")
        nc.sync.dma_start(bqk_sb[:], bqk.rearrange("(c p) -> p c", p=P))
        bvb_sb = cpool.tile([1, HEADS_PER_CORE * VW], F32R, tag="bvbrow")
        nc.sync.dma_start(bvb_sb[:], bvb[None, :])
        for k in range(1, KT // 2):
            nc.sync.dma_start(
                xkt2[k][:],
                xk[2 * k * P:2 * (k + 1) * P, :].rearrange(
                    "(g p) t -> p g t", p=P))
        wq_sb = wpool.tile([P, KT, LOCAL_E], BF16, tag="wq")
        nc.sync.dma_start(wq_sb[:], wqt.rearrange("(k p) n -> p k n", p=P))
        xqn = [xqpool.tile([P, KT, SQ_BLK], BF16, tag="xq", name=f"xq{n}")
               for n in range(NSQ)]
        nc.sync.dma_start(
            xqn[0][:], xq[0:E, :].rearrange("(k p) n -> p k n", p=P))
        wv_sb = wpool.tile([P, KT, LOCAL_E], BF16, tag="wv")
        nc.sync.dma_start(wv_sb[:], wvt.rearrange("(k p) n -> p k n", p=P))
        xvt2 = [xpool.tile([P, 2, T], BF16, tag="x", name=f"xv{k}")
                for k in range(KT // 2)]
        for k in range(KT // 2):
            nc.sync.dma_start(
                xvt2[k][:],
                xv[2 * k * P:2 * (k + 1) * P, :].rearrange(
                    "(g p) t -> p g t", p=P))
        for n in range(1, NSQ):
            nc.sync.dma_start(
                xqn[n][:], xq[n * E:(n + 1) * E, :].rearrange(
                    "(k p) n -> p k n", p=P))
        wo_sb = wpool.tile([P, LOCAL_E // P, E], BF16, tag="wo")
        nc.sync.dma_start(wo_sb[:], wot.rearrange("(k p) n -> p k n", p=P))

        def xk_tile(k):
            return xkt2[k // 2][:, k % 2, :]

        def xv_tile(k):
            return xvt2[k // 2][:, k % 2, :]

        # ---- bvb broadcast across partitions (K=1 matmul) --------------
        bvb_ps = pox.tile([P, 2 * SQ_BLK], F32, tag="px", name="bvbps")
        nc.tensor.matmul(bvb_ps[:, 0:HEADS_PER_CORE * VW],
                         onesr[0:1, :], bvb_sb[0:1, :],
                         start=True, stop=True)
        bvb_bc = cpool.tile([P, HEADS_PER_CORE * VW], F32, tag="bvbbc")
        nc.vector.tensor_copy(bvb_bc[:], bvb_ps[:, 0:HEADS_PER_CORE * VW])

        # ---- persistent activations ------------------------------------
        # pair-stacked: rows 0:64 = head 2p, rows 64:128 = head 2p+1
        kp = [qkpool.tile([P, T], BF16, tag="qk", name=f"kp{i}")
              for i in range(NPAIR)]
        qp = [qkpool.tile([P, T], BF16, tag="qk", name=f"qp{i}")
              for i in range(NPAIR)]
        # v buffer: per sk-tile, per head: [v_h (64 cols) | ones (1 col)]
        vbuf = vpool.tile([P, NSK, HEADS_PER_CORE * VW], BF16, tag="v")
        outU = [opool.tile([P, T], BF16, tag="o", name=f"outU{i}")
                for i in range(NPAIR)]
        outT = [opool.tile([P, T], BF16, tag="o", name=f"outT{i}")
                for i in range(NPAIR)]

        # ---- K projection: k-outer over 8 concurrent PSUM groups ------
        def kq_proj_full(w_sb, xt, bias_col, dst):
            gt = [psc.tile([P, 2 * SQ_BLK], F32, tag="sc", name=f"pg{i}")
                  for i in range(2)]
            gx = pox.tile([P, 2 * SQ_BLK], F32, tag="px", name="pgx")
            gp = [po.tile([P, SQ_BLK], F32, tag="po", name=f"pgp{i}")
                  for i in range(2)]
            groups = [gt[0][:, 0:SQ_BLK], gt[0][:, SQ_BLK:2 * SQ_BLK],
                      gt[1][:, 0:SQ_BLK], gt[1][:, SQ_BLK:2 * SQ_BLK],
                      gx[:, 0:SQ_BLK], gx[:, SQ_BLK:2 * SQ_BLK],
                      gp[0][:, :], gp[1][:, :]]
            for k in range(KT):
                for pr in range(NPAIR):
                    for n in range(NSQ):
                        nc.tensor.matmul(
                            groups[pr * NSQ + n],
                            w_sb[:, k, pr * P:(pr + 1) * P],
                            xt(k)[:, n * SQ_BLK:(n + 1) * SQ_BLK],
                            start=(k == 0), stop=(k == KT - 1))
            for pr in range(NPAIR):
                for n in range(NSQ):
                    nc.vector.tensor_scalar_add(
                        dst[pr][:, n * SQ_BLK:(n + 1) * SQ_BLK],
                        groups[pr * NSQ + n],
                        bqk_sb[:, bias_col + pr:bias_col + pr + 1])

        kq_proj_full(wk_sb, xk_tile, 2, kp)

        # ---- Q projection, one sq-block at a time ----------------------
        def q_proj_block(n):
            ps = pox.tile([P, 2 * SQ_BLK], F32, tag="px", name=f"qpg{n}")
            for pr in range(NPAIR):
                for k in range(KT):
                    nc.tensor.matmul(
                        ps[:, pr * SQ_BLK:(pr + 1) * SQ_BLK],
                        wq_sb[:, k, pr * P:(pr + 1) * P],
                        xqn[n][:, k, :],
                        start=(k == 0), stop=(k == KT - 1))
            for pr in range(NPAIR):
                nc.vector.tensor_scalar_add(
                    qp[pr][:, n * SQ_BLK:(n + 1) * SQ_BLK],
                    ps[:, pr * SQ_BLK:(pr + 1) * SQ_BLK],
                    bqk_sb[:, pr:pr + 1])

        q_proj_block(0)

        # ---- V projection: per token-tile, fills attention PE slack ----
        def v_proj_tile(tt):
            ps_t = pox.tile([P, 2 * SQ_BLK], F32, tag="px", name=f"vp{tt}")
            ps = ps_t[:, 0:LOCAL_E]
            for k in range(KT):
                nc.tensor.matmul(
                    ps,
                    xv_tile(k)[:, tt * P:(tt + 1) * P],
                    wv_sb[:, k, :],
                    start=(k == 0), stop=(k == KT - 1))
            nc.vector.tensor_tensor(
                vbuf.rearrange("p s (h c) -> p s h c", c=VW)[:, tt, :, 0:HD],
                ps.rearrange("p (h c) -> p h c", c=HD),
                bvb_bc.rearrange("p (h c) -> p h c", c=VW)[:, :, 0:HD],
                ADD)
            nc.vector.tensor_copy(
                vbuf.rearrange("p s (h c) -> p s h c", c=VW)
                [:, tt, :, HD:HD + 1],
                bvb_bc.rearrange("p (h c) -> p h c", c=VW)[:, :, HD:HD + 1])

        # V projection and late Q blocks: emitted pre-attention (correct
        # program order for the dependency tracker) but priority-DEMOTED so
        # the scheduler slots them into PE slack of the ACT-paced attention
        # loops instead of running them ahead of the first scores.
        # ~5 instructions per attention sk-iteration.
        # NOTE: pox slot reuse is FIFO in call order, so demotion offsets
        # must keep pox users' priorities monotone in emission order.
        for tt in range(NSK):
            with tc.high_priority(offset=-(3 + 5 * tt)):
                v_proj_tile(tt)
        for n in range(1, NSQ):
            with tc.high_priority(offset=-(131 + 30 * (n - 1))):
                q_proj_block(n)

        def make_normalize_a(sq, pr, po_t, rcr):
            """DVE-only part: denominators + outU eviction (releases po)."""
            sqs = slice(sq * SQ_BLK, (sq + 1) * SQ_BLK)

            def emit():
                dn = spool.tile([1, 2 * SQ_BLK], F32, tag="dn",
                                name=f"dn{pr}_{sq}")
                for X in range(2):
                    nc.vector.tensor_copy(
                        dn[0:1, X * SQ_BLK:(X + 1) * SQ_BLK],
                        po_t[X][HD:VW, :])
                rc = spool.tile([1, 2 * SQ_BLK], F32, tag="rc",
                                name=f"rc{pr}_{sq}")
                nc.vector.reciprocal_approx_fast(rc[:], dn[:])
                nc.vector.tensor_copy(rcr[:], rc[:])
                for X in range(2):
                    nc.vector.tensor_copy(
                        outU[pr][X * HD:(X + 1) * HD, sqs], po_t[X][0:HD, :])
            return emit

        def make_normalize_b(sq, pr, rcr):
            """PE broadcast + multiply (outT)."""
            sqs = slice(sq * SQ_BLK, (sq + 1) * SQ_BLK)

            def emit():
                pb = pox.tile([P, 2 * SQ_BLK], F32, tag="px",
                              name=f"pb{pr}_{sq}")
                for X in range(2):
                    nc.tensor.matmul(pb[:, X * SQ_BLK:(X + 1) * SQ_BLK],
                                     onesr[0:1, :],
                                     rcr[0:1, X * SQ_BLK:(X + 1) * SQ_BLK],
                                     start=True, stop=True)
                    nc.vector.tensor_tensor(
                        outT[pr][X * HD:(X + 1) * HD, sqs],
                        outU[pr][X * HD:(X + 1) * HD, sqs],
                        pb[X * HD:(X + 1) * HD, X * SQ_BLK:(X + 1) * SQ_BLK],
                        MULT)
            return emit

        def make_oproj(sq, mp, pool):
            sqs = slice(sq * SQ_BLK, (sq + 1) * SQ_BLK)

            def emit():
                pst = pool.tile([P, 2 * SQ_BLK], F32,
                                tag="sc" if pool is psc else "px",
                                name=f"op{sq}_{mp}")
                for half in range(2):
                    m = 2 * mp + half
                    for kb in range(NPAIR):
                        nc.tensor.matmul(
                            pst[:, half * SQ_BLK:(half + 1) * SQ_BLK],
                            wo_sb[:, kb, m * P:(m + 1) * P],
                            outT[kb][:, sqs],
                            start=(kb == 0), stop=(kb == NPAIR - 1))
                fin = fpool.tile([P, 2 * SQ_BLK], F32, tag="f")
                nc.vector.tensor_copy(fin[:], pst[:])
                nc.sync.dma_start(
                    outp[mp * 2 * P:(mp + 1) * 2 * P, sqs].rearrange(
                        "(g p) n -> p g n", p=P),
                    fin.rearrange("p (g n) -> p g n", g=2))
            return emit

        # ---- attention: normalize/o-proj deferred into the next pair's
        # loop; fill units popped every other iteration ------------------
        pending = []
        for sq in range(NSQ):
            sqs = slice(sq * SQ_BLK, (sq + 1) * SQ_BLK)
            for pr in range(NPAIR):
                po_t = [po.tile([P, SQ_BLK], F32, tag="po",
                                name=f"po{pr}_{sq}_{i}") for i in range(2)]
                for sk in range(NSK):
                    sks = slice(sk * P, (sk + 1) * P)
                    ps = psc.tile([P, 2 * SQ_BLK], F32, tag="sc")
                    # both heads' scoresT concurrently (PE row-tiles T0/T8)
                    nc.tensor.matmul(ps[:, 0:SQ_BLK],
                                     kp[pr][0:HD, sks], qp[pr][0:HD, sqs],
                                     start=True, stop=True)
                    nc.tensor.matmul(ps[:, SQ_BLK:2 * SQ_BLK],
                                     kp[pr][HD:P, sks], qp[pr][HD:P, sqs],
                                     start=True, stop=True)
                    ex = epool.tile([P, 2 * SQ_BLK], BF16, tag="e")
                    nc.scalar.activation(ex[:], ps[:], EXPF)
                    for X in range(2):
                        h = 2 * pr + X
                        nc.tensor.matmul(
                            po_t[X][0:VW, :],
                            vbuf[:, sk, h * VW:(h + 1) * VW],
                            ex[:, X * SQ_BLK:(X + 1) * SQ_BLK],
                            start=(sk == 0), stop=(sk == NSK - 1))
                    # deferred consumers from the previous pair / sq
                    if sk == 1 and pending:
                        pending.pop(0)()
                    elif sk >= 4 and sk % 2 == 0 and pending:
                        pending.pop(0)()
                rcr = spool.tile([1, 2 * SQ_BLK], F32R, tag="rcr",
                                 name=f"rcr{pr}_{sq}")
                pending.append(make_normalize_a(sq, pr, po_t, rcr))
                pending.append(make_normalize_b(sq, pr, rcr))
            for mp in range(E // (2 * P)):
                # final sq's o-proj alternates pools so the tail pipelines
                pool = psc if (sq == NSQ - 1 and mp % 2 == 0) else pox
                pending.append(make_oproj(sq, mp, pool))
        for p in pending:
            p()

    nc.compile()
    return nc


_NC = None


def _get_nc():
    global _NC
    if _NC is None:
        _NC = _build_program()
    return _NC


def _bf(a):
    return np.ascontiguousarray(a.astype(NPBF16))


def _make_in_maps(inputs):
    q = np.asarray(inputs["query"], np.float32)
    k = np.asarray(inputs["key"], np.float32)
    v = np.asarray(inputs["value"], np.float32)
    Wq = np.asarray(inputs["Wq"], np.float32)
    Wk = np.asarray(inputs["Wk"], np.float32)
    Wv = np.asarray(inputs["Wv"], np.float32)
    Wo = np.asarray(inputs["Wo"], np.float32)
    bq = np.asarray(inputs["bq"], np.float32)
    bk = np.asarray(inputs["bk"], np.float32)
    bv = np.asarray(inputs["bv"], np.float32)
    scale = np.float32(HD ** -0.5)

    in_maps = []
    for c in range(NCORES):
        b = c // CORES_PER_BATCH
        h0 = (c % CORES_PER_BATCH) * HEADS_PER_CORE
        hsl = slice(h0 * HD, (h0 + HEADS_PER_CORE) * HD)
        bvh = bv[hsl].reshape(HEADS_PER_CORE, HD)
        bvb = np.concatenate(
            [bvh, np.ones((HEADS_PER_CORE, 1), np.float32)], axis=1).ravel()
        xqT = q[:, b, :].T                                  # [E, T]
        xq_blocks = np.concatenate(
            [xqT[:, n * SQ_BLK:(n + 1) * SQ_BLK] for n in range(NSQ)], axis=0)
        in_maps.append({
            "xq": _bf(xq_blocks),
            "xk": _bf(k[:, b, :].T),
            "xv": _bf(v[:, b, :].T),
            "wqt": _bf((Wq[hsl, :] * scale).T),
            "wkt": _bf(Wk[hsl, :].T),
            "wvt": _bf(Wv[hsl, :].T),
            "wot": _bf(Wo[:, hsl].T),
            "bqk": np.ascontiguousarray(
                np.concatenate([bq[hsl] * scale, bk[hsl]]).astype(np.float32)),
            "bvb": np.ascontiguousarray(bvb.astype(np.float32)),
        })
    return in_maps


def run_sharded(inputs, trace=False):
    """Returns (full_output [S,B,E] f32, BassKernelResults)."""
    nc = _get_nc()
    in_maps = _make_in_maps(inputs)
    res = run_bass_kernel_spmd(nc, in_maps, core_ids=list(range(NCORES)),
                               trace=trace)
    bo = np.asarray(inputs["bo"], np.float32)
    final = np.zeros((S, B, E), np.float32)
    for c in range(NCORES):
        b = c // CORES_PER_BATCH
        final[:, b, :] += res.results[c]["outp"].T
    final += bo
    return final, res


def kernel(**inputs):
    out, _ = run_sharded(inputs, trace=False)
    return out


# revision 62
# speedup vs baseline: 1.2211x; 1.0187x over previous
"""Multi-head attention (S=2048, B=2, E=1024, H=16) on 8 Trainium2 cores.

Sharding: data-parallel over batch (4 cores per batch element) x tensor-parallel
over heads (4 heads per core), Megatron-style: Wq/Wk/Wv column-sharded,
Wo row-sharded, per-core partial outputs summed (+bo) on host.

V2 (vs V1 baseline at 295.7us):
- q/k/outU/outT stored PAIR-STACKED [128, T] (head 2p rows 0:64, head 2p+1 rows
  64:128). Score matmuls for the two heads of a pair run CONCURRENTLY as PE
  row-tiles T0/T8 (tile_position (0,0)/(64,0), auto-derived from base
  partitions): one 512-cycle streaming window computes both heads' scores.
- PE warmup: ~40 dummy matmuls on a memset tile at kernel start flip the HAM
  clock gate to 8/8 before real matmuls arrive (baseline ran its first ~39us
  at 1.2GHz).
- Constants built with DVE memset (no DMA); biases in 3 small DMAs issued
  after the first xk tile.
- DMA order: wk, xk (k-tiles), wq, xq-n0-block, wv, xv, xq-n1..3, wo. xq is
  staged in sq-block-major dram layout so the first scores only need
  wk+xk+wq+xq_n0 (6MB) instead of 9MB.
- exp run-ahead: 16 ex buffers decouple ACT (the inner-loop pacer, 1147ns per
  [128,1024] exp) from lagging PV matmuls (vbuf lands late); V-projection and
  late Q-projection matmuls fill PE slack inside the ACT-paced loops via the
  readiness-driven Tile scheduler.
- Normalize: one K=2 broadcast matmul per pair (selector lhsT) broadcasts both
  heads' 1/denom rows across partitions; outT = outU * pb multiplies straight
  from PSUM. Softmax denominators still ride row 64 of the PV matmuls (M=65).
- o-projection evicted as [128,1024] tiles (2 m-blocks per PSUM tile/eviction/
  DMA).
PSUM: psc 3x[128,1024] (6 banks: score/exp double-buffer + scratch for v-proj,
o-proj, bcast, projections) + po 2x[128,512] (PV accumulators; also the last 2
projection groups). K/Q projections run k-outer across 8 concurrent groups.
"""

import numpy as np
import ml_dtypes
from contextlib import ExitStack

import concourse.bass as bass
import concourse.mybir as mybir
from concourse import bacc
import concourse.tile as tile
from concourse.bass_utils import run_bass_kernel_spmd

S, B, E, H, HD = 2048, 2, 1024, 16, 64
P = 128
NCORES = 8
CORES_PER_BATCH = 4
HEADS_PER_CORE = H // CORES_PER_BATCH      # 4
LOCAL_E = HEADS_PER_CORE * HD              # 256
VW = HD + 1                                # 65: [v_h | ones]
T = S                                      # tokens per core (one batch elem)
KT = E // P                                # 8 contraction tiles for projections
NPAIR = HEADS_PER_CORE // 2                # 2 head pairs
SQ_BLK = 512
NSQ = T // SQ_BLK                          # 4
NSK = T // P                               # 16
F32 = mybir.dt.float32
F32R = mybir.dt.float32r
BF16 = mybir.dt.bfloat16
NPBF16 = ml_dtypes.bfloat16
EXPF = mybir.ActivationFunctionType.Exp
ADD = mybir.AluOpType.add
MULT = mybir.AluOpType.mult

N_WARMUP = 40


def _build_program():
    nc = bacc.Bacc("TRN2")

    xk = nc.dram_tensor("xk", [E, T], BF16, kind="ExternalInput")
    # xq staged sq-block-major: block n holds xT[:, n*512:(n+1)*512]
    xq = nc.dram_tensor("xq", [NSQ * E, SQ_BLK], BF16, kind="ExternalInput")
    xv = nc.dram_tensor("xv", [E, T], BF16, kind="ExternalInput")
    wqt = nc.dram_tensor("wqt", [E, LOCAL_E], BF16, kind="ExternalInput")
    wkt = nc.dram_tensor("wkt", [E, LOCAL_E], BF16, kind="ExternalInput")
    wvt = nc.dram_tensor("wvt", [E, LOCAL_E], BF16, kind="ExternalInput")
    wot = nc.dram_tensor("wot", [LOCAL_E, E], BF16, kind="ExternalInput")
    # [bq_pair0 | bq_pair1 | bk_pair0 | bk_pair1], 128 rows each
    bqk = nc.dram_tensor("bqk", [2 * LOCAL_E], F32, kind="ExternalInput")
    # per head: [bv_head (64), 1.0] -> 65 columns
    bvb = nc.dram_tensor("bvb", [HEADS_PER_CORE * VW], F32R,
                         kind="ExternalInput")
    outp = nc.dram_tensor("outp", [E, T], BF16, kind="ExternalOutput")

    with ExitStack() as ctx:
        ctx.enter_context(nc.allow_low_precision(reason="bf16 matmul pipeline"))
        tc = ctx.enter_context(tile.TileContext(nc))
        cpool = ctx.enter_context(tc.tile_pool(name="cpool", bufs=1))
        spool = ctx.enter_context(tc.tile_pool(name="spool", bufs=2))
        xpool = ctx.enter_context(tc.tile_pool(name="xpool", bufs=6))
        xqpool = ctx.enter_context(tc.tile_pool(name="xqpool", bufs=4))
        wpool = ctx.enter_context(tc.tile_pool(name="wpool", bufs=1))
        qkpool = ctx.enter_context(tc.tile_pool(name="qkpool", bufs=4))
        vpool = ctx.enter_context(tc.tile_pool(name="vpool", bufs=1))
        opool = ctx.enter_context(tc.tile_pool(name="opool", bufs=4))
        epool = ctx.enter_context(tc.tile_pool(name="epool", bufs=16))
        fpool = ctx.enter_context(tc.tile_pool(name="fpool", bufs=4))
        # PSUM (8 banks): psc 2x[128,1024] (score/exp double-buffer, 4 banks)
        # + pox 2x[128,512] (projection/normalize/o-proj scratch, 2 banks,
        # double-buffered so evictions overlap the next unit's matmuls)
        # + po 2x[128,512] (PV accumulators, 2 banks)
        psc = ctx.enter_context(tc.tile_pool(name="psc", bufs=2, space="PSUM"))
        pox = ctx.enter_context(tc.tile_pool(name="pox", bufs=2, space="PSUM"))
        po = ctx.enter_context(tc.tile_pool(name="po", bufs=2, space="PSUM"))

        # ---- memset constants (no DMA) ---------------------------------
        wtile = cpool.tile([P, P], BF16, tag="warm")
        nc.vector.memset(wtile[:], 0.0)
        onesf = cpool.tile([1, P], F32, tag="onesf")
        nc.vector.memset(onesf[:], 1.0)
        onesr = cpool.tile([1, P], F32R, tag="onesr")
        nc.vector.tensor_copy(onesr[:], onesf[:])

        # ---- PE warmup: flip HAM to 8/8 during the DMA lead-in ---------
        wps = pox.tile([P, SQ_BLK], F32, tag="px", name="warmps")
        for i in range(N_WARMUP):
            nc.tensor.matmul(wps[:, 0:P], wtile[:], wtile[:],
                             start=True, stop=True)

        # ---- DMAs in need-order (few, large) ---------------------------
        # x tiles hold 2 k-tiles each (1MB transfers)
        wk_sb = wpool.tile([P, KT, LOCAL_E], BF16, tag="wk")
        nc.sync.dma_start(wk_sb[:], wkt.rearrange("(k p) n -> p k n", p=P))
        xkt2 = [xpool.tile([P, 2, T], BF16, tag="x", name=f"xk{k}")
                for k in range(KT // 2)]
        nc.sync.dma_start(
            xkt2[0][:], xk[0:2 * P, :].rearrange("(g p) t -> p g t", p=P))
        bqk_sb = cpool.tile([P, 4], F32, tag="bqk")
        nc.sync.dma_start(bqk_sb[:], bqk.rearrange("(c p) -> p c", p=P))
        bvb_sb = cpool.tile([1, HEADS_PER_CORE * VW], F32R, tag="bvbrow")
        nc.sync.dma_start(bvb_sb[:], bvb[None, :])
        for k in range(1, KT // 2):
            nc.sync.dma_start(
                xkt2[k][:],
                xk[2 * k * P:2 * (k + 1) * P, :].rearrange(
                    "(g p) t -> p g t", p=P))
        wq_sb = wpool.tile([P, KT, LOCAL_E], BF16, tag="wq")
        nc.sync.dma_start(wq_sb[:], wqt.rearrange("(k p) n -> p k n", p=P))
        xqn = [xqpool.tile([P, KT, SQ_BLK], BF16, tag="xq", name=f"xq{n}")
               for n in range(NSQ)]
        nc.sync.dma_start(
            xqn[0][:], xq[0:E, :].rearrange("(k p) n -> p k n", p=P))
        wv_sb = wpool.tile([P, KT, LOCAL_E], BF16, tag="wv")
        nc.sync.dma_start(wv_sb[:], wvt.rearrange("(k p) n -> p k n", p=P))
        xvt2 = [xpool.tile([P, 2, T], BF16, tag="x", name=f"xv{k}")
                for k in range(KT // 2)]
        for k in range(KT // 2):
            nc.sync.dma_start(
                xvt2[k][:],
                xv[2 * k * P:2 * (k + 1) * P, :].rearrange(
                    "(g p) t -> p g t", p=P))
        for n in range(1, NSQ):
            nc.sync.dma_start(
                xqn[n][:], xq[n * E:(n + 1) * E, :].rearrange(
                    "(k p) n -> p k n", p=P))
        wo_sb = wpool.tile([P, LOCAL_E // P, E], BF16, tag="wo")
        nc.sync.dma_start(wo_sb[:], wot.rearrange("(k p) n -> p k n", p=P))

        def xk_tile(k):
            return xkt2[k // 2][:, k % 2, :]

        def xv_tile(k):
            return xvt2[k // 2][:, k % 2, :]

        # ---- bvb broadcast across partitions (K=1 matmul) --------------
        bvb_ps = pox.tile([P, SQ_BLK], F32, tag="px", name="bvbps")
        nc.tensor.matmul(bvb_ps[:, 0:HEADS_PER_CORE * VW],
                         onesr[0:1, :], bvb_sb[0:1, :],
                         start=True, stop=True)
        bvb_bc = cpool.tile([P, HEADS_PER_CORE * VW], F32, tag="bvbbc")
        nc.vector.tensor_copy(bvb_bc[:], bvb_ps[:, 0:HEADS_PER_CORE * VW])

        # ---- persistent activations ------------------------------------
        # pair-stacked: rows 0:64 = head 2p, rows 64:128 = head 2p+1
        kp = [qkpool.tile([P, T], BF16, tag="qk", name=f"kp{i}")
              for i in range(NPAIR)]
        qp = [qkpool.tile([P, T], BF16, tag="qk", name=f"qp{i}")
              for i in range(NPAIR)]
        # v buffer: per sk-tile, per head: [v_h (64 cols) | ones (1 col)]
        vbuf = vpool.tile([P, NSK, HEADS_PER_CORE * VW], BF16, tag="v")
        outU = [opool.tile([P, T], BF16, tag="o", name=f"outU{i}")
                for i in range(NPAIR)]
        outT = [opool.tile([P, T], BF16, tag="o", name=f"outT{i}")
                for i in range(NPAIR)]

        # ---- K projection: k-outer over 8 concurrent PSUM groups ------
        def kq_proj_full(w_sb, xt, bias_col, dst):
            gt = [psc.tile([P, 2 * SQ_BLK], F32, tag="sc", name=f"pg{i}")
                  for i in range(2)]
            gx = [pox.tile([P, SQ_BLK], F32, tag="px", name=f"pgx{i}")
                  for i in range(2)]
            gp = [po.tile([P, SQ_BLK], F32, tag="po", name=f"pgp{i}")
                  for i in range(2)]
            groups = [gt[0][:, 0:SQ_BLK], gt[0][:, SQ_BLK:2 * SQ_BLK],
                      gt[1][:, 0:SQ_BLK], gt[1][:, SQ_BLK:2 * SQ_BLK],
                      gx[0][:, :], gx[1][:, :],
                      gp[0][:, :], gp[1][:, :]]
            for k in range(KT):
                for pr in range(NPAIR):
                    for n in range(NSQ):
                        nc.tensor.matmul(
                            groups[pr * NSQ + n],
                            w_sb[:, k, pr * P:(pr + 1) * P],
                            xt(k)[:, n * SQ_BLK:(n + 1) * SQ_BLK],
                            start=(k == 0), stop=(k == KT - 1))
            for pr in range(NPAIR):
                for n in range(NSQ):
                    nc.vector.tensor_scalar_add(
                        dst[pr][:, n * SQ_BLK:(n + 1) * SQ_BLK],
                        groups[pr * NSQ + n],
                        bqk_sb[:, bias_col + pr:bias_col + pr + 1])

        kq_proj_full(wk_sb, xk_tile, 2, kp)

        # ---- Q projection, one sq-block at a time ----------------------
        def q_proj_block(n):
            for pr in range(NPAIR):
                ps = pox.tile([P, SQ_BLK], F32, tag="px",
                              name=f"qpg{n}_{pr}")
                for k in range(KT):
                    nc.tensor.matmul(
                        ps[:, :],
                        wq_sb[:, k, pr * P:(pr + 1) * P],
                        xqn[n][:, k, :],
                        start=(k == 0), stop=(k == KT - 1))
                nc.vector.tensor_scalar_add(
                    qp[pr][:, n * SQ_BLK:(n + 1) * SQ_BLK],
                    ps[:, :], bqk_sb[:, pr:pr + 1])

        q_proj_block(0)

        # ---- V projection: per token-tile, fills attention PE slack ----
        def v_proj_tile(tt):
            ps_t = pox.tile([P, SQ_BLK], F32, tag="px", name=f"vp{tt}")
            ps = ps_t[:, 0:LOCAL_E]
            for k in range(KT):
                nc.tensor.matmul(
                    ps,
                    xv_tile(k)[:, tt * P:(tt + 1) * P],
                    wv_sb[:, k, :],
                    start=(k == 0), stop=(k == KT - 1))
            nc.vector.tensor_tensor(
                vbuf.rearrange("p s (h c) -> p s h c", c=VW)[:, tt, :, 0:HD],
                ps.rearrange("p (h c) -> p h c", c=HD),
                bvb_bc.rearrange("p (h c) -> p h c", c=VW)[:, :, 0:HD],
                ADD)
            nc.vector.tensor_copy(
                vbuf.rearrange("p s (h c) -> p s h c", c=VW)
                [:, tt, :, HD:HD + 1],
                bvb_bc.rearrange("p (h c) -> p h c", c=VW)[:, :, HD:HD + 1])

        # V projection and late Q blocks: emitted pre-attention (correct
        # program order for the dependency tracker) but priority-DEMOTED so
        # the scheduler slots them into PE slack of the ACT-paced attention
        # loops instead of running them ahead of the first scores.
        # ~5 instructions per attention sk-iteration.
        # NOTE: pox slot reuse is FIFO in call order, so demotion offsets
        # must keep pox users' priorities monotone in emission order.
        for tt in range(NSK):
            with tc.high_priority(offset=-(3 + 5 * tt)):
                v_proj_tile(tt)
        for n in range(1, NSQ):
            with tc.high_priority(offset=-(131 + 30 * (n - 1))):
                q_proj_block(n)

        def make_normalize_a(sq, pr, po_t, rcr):
            """DVE-only part: denominators + outU eviction (releases po)."""
            sqs = slice(sq * SQ_BLK, (sq + 1) * SQ_BLK)

            def emit():
                dn = spool.tile([1, 2 * SQ_BLK], F32, tag="dn",
                                name=f"dn{pr}_{sq}")
                for X in range(2):
                    nc.vector.tensor_copy(
                        dn[0:1, X * SQ_BLK:(X + 1) * SQ_BLK],
                        po_t[X][HD:VW, :])
                rc = spool.tile([1, 2 * SQ_BLK], F32, tag="rc",
                                name=f"rc{pr}_{sq}")
                nc.vector.reciprocal_approx_fast(rc[:], dn[:])
                nc.vector.tensor_copy(rcr[:], rc[:])
                for X in range(2):
                    nc.vector.tensor_copy(
                        outU[pr][X * HD:(X + 1) * HD, sqs], po_t[X][0:HD, :])
            return emit

        def make_normalize_b(sq, pr, rcr):
            """PE broadcast + multiply (outT)."""
            sqs = slice(sq * SQ_BLK, (sq + 1) * SQ_BLK)

            def emit():
                for X in range(2):
                    pb = pox.tile([P, SQ_BLK], F32, tag="px",
                                  name=f"pb{pr}_{sq}_{X}")
                    nc.tensor.matmul(pb[:, :],
                                     onesr[0:1, :],
                                     rcr[0:1, X * SQ_BLK:(X + 1) * SQ_BLK],
                                     start=True, stop=True)
                    nc.vector.tensor_tensor(
                        outT[pr][X * HD:(X + 1) * HD, sqs],
                        outU[pr][X * HD:(X + 1) * HD, sqs],
                        pb[X * HD:(X + 1) * HD, :],
                        MULT)
            return emit

        def make_oproj(sq, m, use_psc=False, scalar_evict=False):
            sqs = slice(sq * SQ_BLK, (sq + 1) * SQ_BLK)

            def emit():
                if use_psc:
                    pst = psc.tile([P, 2 * SQ_BLK], F32, tag="sc",
                                   name=f"op{sq}_{m}")[:, 0:SQ_BLK]
                else:
                    pst = pox.tile([P, SQ_BLK], F32, tag="px",
                                   name=f"op{sq}_{m}")[:, :]
                for kb in range(NPAIR):
                    nc.tensor.matmul(
                        pst,
                        wo_sb[:, kb, m * P:(m + 1) * P],
                        outT[kb][:, sqs],
                        start=(kb == 0), stop=(kb == NPAIR - 1))
                fin = fpool.tile([P, SQ_BLK], BF16, tag="f")
                if scalar_evict:
                    nc.scalar.copy(fin[:], pst)
                else:
                    nc.vector.tensor_copy(fin[:], pst)
                nc.sync.dma_start(outp[m * P:(m + 1) * P, sqs], fin[:])
            return emit

        # ---- attention: normalize/o-proj deferred into the next pair's
        # loop. urgent queue = po-releasing DVE work (pops at sk1);
        # main queue = pb/o-proj units (pop one per iteration from sk2) --
        urgent = []
        pending = []
        for sq in range(NSQ):
            sqs = slice(sq * SQ_BLK, (sq + 1) * SQ_BLK)
            for pr in range(NPAIR):
                po_t = [po.tile([P, SQ_BLK], F32, tag="po",
                                name=f"po{pr}_{sq}_{i}") for i in range(2)]
                for sk in range(NSK):
                    sks = slice(sk * P, (sk + 1) * P)
                    ps = psc.tile([P, 2 * SQ_BLK], F32, tag="sc")
                    # both heads' scoresT concurrently (PE row-tiles T0/T8)
                    nc.tensor.matmul(ps[:, 0:SQ_BLK],
                                     kp[pr][0:HD, sks], qp[pr][0:HD, sqs],
                                     start=True, stop=True)
                    nc.tensor.matmul(ps[:, SQ_BLK:2 * SQ_BLK],
                                     kp[pr][HD:P, sks], qp[pr][HD:P, sqs],
                                     start=True, stop=True)
                    ex = epool.tile([P, 2 * SQ_BLK], BF16, tag="e")
                    nc.scalar.activation(ex[:], ps[:], EXPF)
                    for X in range(2):
                        h = 2 * pr + X
                        nc.tensor.matmul(
                            po_t[X][0:VW, :],
                            vbuf[:, sk, h * VW:(h + 1) * VW],
                            ex[:, X * SQ_BLK:(X + 1) * SQ_BLK],
                            start=(sk == 0), stop=(sk == NSK - 1))
                    # deferred consumers from the previous pair / sq
                    if sk == 1 and urgent:
                        urgent.pop(0)()
                    elif sk >= 2 and pending:
                        pending.pop(0)()
                rcr = spool.tile([1, 2 * SQ_BLK], F32R, tag="rcr",
                                 name=f"rcr{pr}_{sq}")
                urgent.append(make_normalize_a(sq, pr, po_t, rcr))
                pending.append(make_normalize_b(sq, pr, rcr))
            last = sq == NSQ - 1
            for m in range(E // P):
                # final sq's o-proj alternates pools + eviction engines so
                # the tail pipelines with no idle engine
                pending.append(make_oproj(sq, m, use_psc=(last and m % 2 == 0),
                                          scalar_evict=(last and m % 2 == 1)))
        for u in urgent:
            u()
        for p in pending:
            p()

    nc.compile()
    return nc


_NC = None


def _get_nc():
    global _NC
    if _NC is None:
        _NC = _build_program()
    return _NC


def _bf(a):
    return np.ascontiguousarray(a.astype(NPBF16))


def _make_in_maps(inputs):
    q = np.asarray(inputs["query"], np.float32)
    k = np.asarray(inputs["key"], np.float32)
    v = np.asarray(inputs["value"], np.float32)
    Wq = np.asarray(inputs["Wq"], np.float32)
    Wk = np.asarray(inputs["Wk"], np.float32)
    Wv = np.asarray(inputs["Wv"], np.float32)
    Wo = np.asarray(inputs["Wo"], np.float32)
    bq = np.asarray(inputs["bq"], np.float32)
    bk = np.asarray(inputs["bk"], np.float32)
    bv = np.asarray(inputs["bv"], np.float32)
    scale = np.float32(HD ** -0.5)

    in_maps = []
    for c in range(NCORES):
        b = c // CORES_PER_BATCH
        h0 = (c % CORES_PER_BATCH) * HEADS_PER_CORE
        hsl = slice(h0 * HD, (h0 + HEADS_PER_CORE) * HD)
        bvh = bv[hsl].reshape(HEADS_PER_CORE, HD)
        bvb = np.concatenate(
            [bvh, np.ones((HEADS_PER_CORE, 1), np.float32)], axis=1).ravel()
        xqT = q[:, b, :].T                                  # [E, T]
        xq_blocks = np.concatenate(
            [xqT[:, n * SQ_BLK:(n + 1) * SQ_BLK] for n in range(NSQ)], axis=0)
        in_maps.append({
            "xq": _bf(xq_blocks),
            "xk": _bf(k[:, b, :].T),
            "xv": _bf(v[:, b, :].T),
            "wqt": _bf((Wq[hsl, :] * scale).T),
            "wkt": _bf(Wk[hsl, :].T),
            "wvt": _bf(Wv[hsl, :].T),
            "wot": _bf(Wo[:, hsl].T),
            "bqk": np.ascontiguousarray(
                np.concatenate([bq[hsl] * scale, bk[hsl]]).astype(np.float32)),
            "bvb": np.ascontiguousarray(bvb.astype(np.float32)),
        })
    return in_maps


def run_sharded(inputs, trace=False):
    """Returns (full_output [S,B,E] f32, BassKernelResults)."""
    nc = _get_nc()
    in_maps = _make_in_maps(inputs)
    res = run_bass_kernel_spmd(nc, in_maps, core_ids=list(range(NCORES)),
                               trace=trace)
    bo = np.asarray(inputs["bo"], np.float32)
    final = np.zeros((S, B, E), np.float32)
    for c in range(NCORES):
        b = c // CORES_PER_BATCH
        final[:, b, :] += res.results[c]["outp"].astype(np.float32).T
    final += bo
    return final, res


def kernel(**inputs):
    out, _ = run_sharded(inputs, trace=False)
    return out


# revision 63
# speedup vs baseline: 1.2790x; 1.0474x over previous
"""Multi-head attention (S=2048, B=2, E=1024, H=16) on 8 Trainium2 cores.

Sharding: data-parallel over batch (4 cores per batch element) x tensor-parallel
over heads (4 heads per core), Megatron-style: Wq/Wk/Wv column-sharded,
Wo row-sharded, per-core partial outputs summed (+bo) on host.

V2 (vs V1 baseline at 295.7us):
- q/k/outU/outT stored PAIR-STACKED [128, T] (head 2p rows 0:64, head 2p+1 rows
  64:128). Score matmuls for the two heads of a pair run CONCURRENTLY as PE
  row-tiles T0/T8 (tile_position (0,0)/(64,0), auto-derived from base
  partitions): one 512-cycle streaming window computes both heads' scores.
- PE warmup: ~40 dummy matmuls on a memset tile at kernel start flip the HAM
  clock gate to 8/8 before real matmuls arrive (baseline ran its first ~39us
  at 1.2GHz).
- Constants built with DVE memset (no DMA); biases in 3 small DMAs issued
  after the first xk tile.
- DMA order: wk, xk (k-tiles), wq, xq-n0-block, wv, xv, xq-n1..3, wo. xq is
  staged in sq-block-major dram layout so the first scores only need
  wk+xk+wq+xq_n0 (6MB) instead of 9MB.
- exp run-ahead: 16 ex buffers decouple ACT (the inner-loop pacer, 1147ns per
  [128,1024] exp) from lagging PV matmuls (vbuf lands late); V-projection and
  late Q-projection matmuls fill PE slack inside the ACT-paced loops via the
  readiness-driven Tile scheduler.
- Normalize: one K=2 broadcast matmul per pair (selector lhsT) broadcasts both
  heads' 1/denom rows across partitions; outT = outU * pb multiplies straight
  from PSUM. Softmax denominators still ride row 64 of the PV matmuls (M=65).
- o-projection evicted as [128,1024] tiles (2 m-blocks per PSUM tile/eviction/
  DMA).
PSUM: psc 3x[128,1024] (6 banks: score/exp double-buffer + scratch for v-proj,
o-proj, bcast, projections) + po 2x[128,512] (PV accumulators; also the last 2
projection groups). K/Q projections run k-outer across 8 concurrent groups.
"""

import numpy as np
import ml_dtypes
from contextlib import ExitStack

import concourse.bass as bass
import concourse.mybir as mybir
from concourse import bacc
import concourse.tile as tile
from concourse.bass_utils import run_bass_kernel_spmd

S, B, E, H, HD = 2048, 2, 1024, 16, 64
P = 128
NCORES = 8
CORES_PER_BATCH = 4
HEADS_PER_CORE = H // CORES_PER_BATCH      # 4
LOCAL_E = HEADS_PER_CORE * HD              # 256
VW = HD + 1                                # 65: [v_h | ones]
T = S                                      # tokens per core (one batch elem)
KT = E // P                                # 8 contraction tiles for projections
NPAIR = HEADS_PER_CORE // 2                # 2 head pairs
SQ_BLK = 512
NSQ = T // SQ_BLK                          # 4
NSK = T // P                               # 16
F32 = mybir.dt.float32
F32R = mybir.dt.float32r
BF16 = mybir.dt.bfloat16
NPBF16 = ml_dtypes.bfloat16
EXPF = mybir.ActivationFunctionType.Exp
ADD = mybir.AluOpType.add
MULT = mybir.AluOpType.mult

N_WARMUP = 40


def _build_program():
    nc = bacc.Bacc("TRN2")

    xk = nc.dram_tensor("xk", [E, T], BF16, kind="ExternalInput")
    # xq staged sq-block-major: block n holds xT[:, n*512:(n+1)*512]
    xq = nc.dram_tensor("xq", [NSQ * E, SQ_BLK], BF16, kind="ExternalInput")
    xv = nc.dram_tensor("xv", [E, T], BF16, kind="ExternalInput")
    wqt = nc.dram_tensor("wqt", [E, LOCAL_E], BF16, kind="ExternalInput")
    wkt = nc.dram_tensor("wkt", [E, LOCAL_E], BF16, kind="ExternalInput")
    wvt = nc.dram_tensor("wvt", [E, LOCAL_E], BF16, kind="ExternalInput")
    wot = nc.dram_tensor("wot", [LOCAL_E, E], BF16, kind="ExternalInput")
    # [bq_pair0 | bq_pair1 | bk_pair0 | bk_pair1], 128 rows each
    bqk = nc.dram_tensor("bqk", [2 * LOCAL_E], F32, kind="ExternalInput")
    # per head: [bv_head (64), 1.0] -> 65 columns
    bvb = nc.dram_tensor("bvb", [HEADS_PER_CORE * VW], F32R,
                         kind="ExternalInput")
    outp = nc.dram_tensor("outp", [E, T], BF16, kind="ExternalOutput")

    with ExitStack() as ctx:
        ctx.enter_context(nc.allow_low_precision(reason="bf16 matmul pipeline"))
        tc = ctx.enter_context(tile.TileContext(nc))
        cpool = ctx.enter_context(tc.tile_pool(name="cpool", bufs=1))
        spool = ctx.enter_context(tc.tile_pool(name="spool", bufs=2))
        xpool = ctx.enter_context(tc.tile_pool(name="xpool", bufs=6))
        xqpool = ctx.enter_context(tc.tile_pool(name="xqpool", bufs=4))
        wpool = ctx.enter_context(tc.tile_pool(name="wpool", bufs=1))
        qkpool = ctx.enter_context(tc.tile_pool(name="qkpool", bufs=4))
        vpool = ctx.enter_context(tc.tile_pool(name="vpool", bufs=1))
        opool = ctx.enter_context(tc.tile_pool(name="opool", bufs=4))
        epool = ctx.enter_context(tc.tile_pool(name="epool", bufs=16))
        fpool = ctx.enter_context(tc.tile_pool(name="fpool", bufs=4))
        # PSUM (8 banks): psc 2x[128,1024] (score/exp double-buffer, 4 banks)
        # + pox 2x[128,512] (projection/normalize/o-proj scratch, 2 banks,
        # double-buffered so evictions overlap the next unit's matmuls)
        # + po 2x[128,512] (PV accumulators, 2 banks)
        psc = ctx.enter_context(tc.tile_pool(name="psc", bufs=2, space="PSUM"))
        pox = ctx.enter_context(tc.tile_pool(name="pox", bufs=2, space="PSUM"))
        po = ctx.enter_context(tc.tile_pool(name="po", bufs=2, space="PSUM"))

        # ---- memset constants (no DMA) ---------------------------------
        wtile = cpool.tile([P, P], BF16, tag="warm")
        nc.vector.memset(wtile[:], 0.0)
        onesf = cpool.tile([1, P], F32, tag="onesf")
        nc.vector.memset(onesf[:], 1.0)
        onesr = cpool.tile([1, P], F32R, tag="onesr")
        nc.vector.tensor_copy(onesr[:], onesf[:])

        # ---- PE warmup: flip HAM to 8/8 during the DMA lead-in ---------
        wps = pox.tile([P, SQ_BLK], F32, tag="px", name="warmps")
        for i in range(N_WARMUP):
            nc.tensor.matmul(wps[:, 0:P], wtile[:], wtile[:],
                             start=True, stop=True)

        # ---- DMAs in need-order (few, large) ---------------------------
        # x tiles hold 2 k-tiles each (1MB transfers)
        wk_sb = wpool.tile([P, KT, LOCAL_E], BF16, tag="wk")
        nc.scalar.dma_start(wk_sb[:], wkt.rearrange("(k p) n -> p k n", p=P))
        xkt2 = [xpool.tile([P, 2, T], BF16, tag="x", name=f"xk{k}")
                for k in range(KT // 2)]
        nc.sync.dma_start(
            xkt2[0][:], xk[0:2 * P, :].rearrange("(g p) t -> p g t", p=P))
        bqk_sb = cpool.tile([P, 4], F32, tag="bqk")
        nc.scalar.dma_start(bqk_sb[:], bqk.rearrange("(c p) -> p c", p=P))
        bvb_sb = cpool.tile([1, HEADS_PER_CORE * VW], F32R, tag="bvbrow")
        nc.scalar.dma_start(bvb_sb[:], bvb[None, :])
        for k in range(1, KT // 2):
            nc.sync.dma_start(
                xkt2[k][:],
                xk[2 * k * P:2 * (k + 1) * P, :].rearrange(
                    "(g p) t -> p g t", p=P))
        wq_sb = wpool.tile([P, KT, LOCAL_E], BF16, tag="wq")
        nc.scalar.dma_start(wq_sb[:], wqt.rearrange("(k p) n -> p k n", p=P))
        xqn = [xqpool.tile([P, KT, SQ_BLK], BF16, tag="xq", name=f"xq{n}")
               for n in range(NSQ)]
        nc.sync.dma_start(
            xqn[0][:], xq[0:E, :].rearrange("(k p) n -> p k n", p=P))
        wv_sb = wpool.tile([P, KT, LOCAL_E], BF16, tag="wv")
        nc.scalar.dma_start(wv_sb[:], wvt.rearrange("(k p) n -> p k n", p=P))
        xvt2 = [xpool.tile([P, 2, T], BF16, tag="x", name=f"xv{k}")
                for k in range(KT // 2)]
        for k in range(KT // 2):
            nc.sync.dma_start(
                xvt2[k][:],
                xv[2 * k * P:2 * (k + 1) * P, :].rearrange(
                    "(g p) t -> p g t", p=P))
        for n in range(1, NSQ):
            nc.sync.dma_start(
                xqn[n][:], xq[n * E:(n + 1) * E, :].rearrange(
                    "(k p) n -> p k n", p=P))
        wo_sb = wpool.tile([P, LOCAL_E // P, E], BF16, tag="wo")
        nc.scalar.dma_start(wo_sb[:], wot.rearrange("(k p) n -> p k n", p=P))

        def xk_tile(k):
            return xkt2[k // 2][:, k % 2, :]

        def xv_tile(k):
            return xvt2[k // 2][:, k % 2, :]

        # ---- bvb broadcast across partitions (K=1 matmul) --------------
        bvb_ps = pox.tile([P, SQ_BLK], F32, tag="px", name="bvbps")
        nc.tensor.matmul(bvb_ps[:, 0:HEADS_PER_CORE * VW],
                         onesr[0:1, :], bvb_sb[0:1, :],
                         start=True, stop=True)
        bvb_bc = cpool.tile([P, HEADS_PER_CORE * VW], F32, tag="bvbbc")
        nc.vector.tensor_copy(bvb_bc[:], bvb_ps[:, 0:HEADS_PER_CORE * VW])

        # ---- persistent activations ------------------------------------
        # pair-stacked: rows 0:64 = head 2p, rows 64:128 = head 2p+1
        kp = [qkpool.tile([P, T], BF16, tag="qk", name=f"kp{i}")
              for i in range(NPAIR)]
        qp = [qkpool.tile([P, T], BF16, tag="qk", name=f"qp{i}")
              for i in range(NPAIR)]
        # v buffer: per sk-tile, per head: [v_h (64 cols) | ones (1 col)]
        vbuf = vpool.tile([P, NSK, HEADS_PER_CORE * VW], BF16, tag="v")
        outU = [opool.tile([P, T], BF16, tag="o", name=f"outU{i}")
                for i in range(NPAIR)]
        outT = [opool.tile([P, T], BF16, tag="o", name=f"outT{i}")
                for i in range(NPAIR)]

        # ---- K projection: k-outer over 8 concurrent PSUM groups ------
        def kq_proj_full(w_sb, xt, bias_col, dst):
            gt = [psc.tile([P, 2 * SQ_BLK], F32, tag="sc", name=f"pg{i}")
                  for i in range(2)]
            gx = [pox.tile([P, SQ_BLK], F32, tag="px", name=f"pgx{i}")
                  for i in range(2)]
            gp = [po.tile([P, SQ_BLK], F32, tag="po", name=f"pgp{i}")
                  for i in range(2)]
            groups = [gt[0][:, 0:SQ_BLK], gt[0][:, SQ_BLK:2 * SQ_BLK],
                      gt[1][:, 0:SQ_BLK], gt[1][:, SQ_BLK:2 * SQ_BLK],
                      gx[0][:, :], gx[1][:, :],
                      gp[0][:, :], gp[1][:, :]]
            for k in range(KT):
                for pr in range(NPAIR):
                    for n in range(NSQ):
                        nc.tensor.matmul(
                            groups[pr * NSQ + n],
                            w_sb[:, k, pr * P:(pr + 1) * P],
                            xt(k)[:, n * SQ_BLK:(n + 1) * SQ_BLK],
                            start=(k == 0), stop=(k == KT - 1))
            for pr in range(NPAIR):
                for n in range(NSQ):
                    nc.vector.tensor_scalar_add(
                        dst[pr][:, n * SQ_BLK:(n + 1) * SQ_BLK],
                        groups[pr * NSQ + n],
                        bqk_sb[:, bias_col + pr:bias_col + pr + 1])

        kq_proj_full(wk_sb, xk_tile, 2, kp)

        # ---- Q projection, one sq-block at a time ----------------------
        def q_proj_block(n):
            for pr in range(NPAIR):
                ps = pox.tile([P, SQ_BLK], F32, tag="px",
                              name=f"qpg{n}_{pr}")
                for k in range(KT):
                    nc.tensor.matmul(
                        ps[:, :],
                        wq_sb[:, k, pr * P:(pr + 1) * P],
                        xqn[n][:, k, :],
                        start=(k == 0), stop=(k == KT - 1))
                nc.vector.tensor_scalar_add(
                    qp[pr][:, n * SQ_BLK:(n + 1) * SQ_BLK],
                    ps[:, :], bqk_sb[:, pr:pr + 1])

        q_proj_block(0)

        # ---- V projection: per token-tile, fills attention PE slack ----
        def v_proj_tile(tt):
            ps_t = pox.tile([P, SQ_BLK], F32, tag="px", name=f"vp{tt}")
            ps = ps_t[:, 0:LOCAL_E]
            for k in range(KT):
                nc.tensor.matmul(
                    ps,
                    xv_tile(k)[:, tt * P:(tt + 1) * P],
                    wv_sb[:, k, :],
                    start=(k == 0), stop=(k == KT - 1))
            nc.vector.tensor_tensor(
                vbuf.rearrange("p s (h c) -> p s h c", c=VW)[:, tt, :, 0:HD],
                ps.rearrange("p (h c) -> p h c", c=HD),
                bvb_bc.rearrange("p (h c) -> p h c", c=VW)[:, :, 0:HD],
                ADD)
            nc.vector.tensor_copy(
                vbuf.rearrange("p s (h c) -> p s h c", c=VW)
                [:, tt, :, HD:HD + 1],
                bvb_bc.rearrange("p (h c) -> p h c", c=VW)[:, :, HD:HD + 1])

        # V projection and late Q blocks: emitted pre-attention (correct
        # program order for the dependency tracker) but priority-DEMOTED so
        # the scheduler slots them into PE slack of the ACT-paced attention
        # loops instead of running them ahead of the first scores.
        # ~5 instructions per attention sk-iteration.
        # NOTE: pox slot reuse is FIFO in call order, so demotion offsets
        # must keep pox users' priorities monotone in emission order.
        for tt in range(NSK):
            with tc.high_priority(offset=-(40 + 6 * tt)):
                v_proj_tile(tt)
        for n in range(1, NSQ):
            with tc.high_priority(offset=-(131 + 30 * (n - 1))):
                q_proj_block(n)

        def make_normalize_a(sq, pr, po_t, rcr):
            """DVE-only part: denominators + outU eviction (releases po)."""
            sqs = slice(sq * SQ_BLK, (sq + 1) * SQ_BLK)

            def emit():
                dn = spool.tile([1, 2 * SQ_BLK], F32, tag="dn",
                                name=f"dn{pr}_{sq}")
                for X in range(2):
                    nc.vector.tensor_copy(
                        dn[0:1, X * SQ_BLK:(X + 1) * SQ_BLK],
                        po_t[X][HD:VW, :])
                rc = spool.tile([1, 2 * SQ_BLK], F32, tag="rc",
                                name=f"rc{pr}_{sq}")
                nc.vector.reciprocal_approx_fast(rc[:], dn[:])
                nc.vector.tensor_copy(rcr[:], rc[:])
                for X in range(2):
                    nc.vector.tensor_copy(
                        outU[pr][X * HD:(X + 1) * HD, sqs], po_t[X][0:HD, :])
            return emit

        def make_normalize_b(sq, pr, rcr):
            """PE broadcast + multiply (outT)."""
            sqs = slice(sq * SQ_BLK, (sq + 1) * SQ_BLK)

            def emit():
                for X in range(2):
                    pb = pox.tile([P, SQ_BLK], F32, tag="px",
                                  name=f"pb{pr}_{sq}_{X}")
                    nc.tensor.matmul(pb[:, :],
                                     onesr[0:1, :],
                                     rcr[0:1, X * SQ_BLK:(X + 1) * SQ_BLK],
                                     start=True, stop=True)
                    nc.vector.tensor_tensor(
                        outT[pr][X * HD:(X + 1) * HD, sqs],
                        outU[pr][X * HD:(X + 1) * HD, sqs],
                        pb[X * HD:(X + 1) * HD, :],
                        MULT)
            return emit

        def make_oproj(sq, m, use_psc=False, scalar_evict=False):
            sqs = slice(sq * SQ_BLK, (sq + 1) * SQ_BLK)

            def emit():
                if use_psc:
                    pst = psc.tile([P, 2 * SQ_BLK], F32, tag="sc",
                                   name=f"op{sq}_{m}")[:, 0:SQ_BLK]
                else:
                    pst = pox.tile([P, SQ_BLK], F32, tag="px",
                                   name=f"op{sq}_{m}")[:, :]
                for kb in range(NPAIR):
                    nc.tensor.matmul(
                        pst,
                        wo_sb[:, kb, m * P:(m + 1) * P],
                        outT[kb][:, sqs],
                        start=(kb == 0), stop=(kb == NPAIR - 1))
                fin = fpool.tile([P, SQ_BLK], BF16, tag="f")
                if scalar_evict:
                    nc.scalar.copy(fin[:], pst)
                else:
                    nc.vector.tensor_copy(fin[:], pst)
                nc.sync.dma_start(outp[m * P:(m + 1) * P, sqs], fin[:])
            return emit

        # ---- attention: normalize/o-proj deferred into the next pair's
        # loop. urgent queue = po-releasing DVE work (pops at sk1);
        # main queue = pb/o-proj units (pop one per iteration from sk2) --
        urgent = []
        pending = []
        for sq in range(NSQ):
            sqs = slice(sq * SQ_BLK, (sq + 1) * SQ_BLK)
            for pr in range(NPAIR):
                po_t = [po.tile([P, SQ_BLK], F32, tag="po",
                                name=f"po{pr}_{sq}_{i}") for i in range(2)]
                for sk in range(NSK):
                    sks = slice(sk * P, (sk + 1) * P)
                    ps = psc.tile([P, 2 * SQ_BLK], F32, tag="sc")
                    # both heads' scoresT concurrently (PE row-tiles T0/T8)
                    nc.tensor.matmul(ps[:, 0:SQ_BLK],
                                     kp[pr][0:HD, sks], qp[pr][0:HD, sqs],
                                     start=True, stop=True)
                    nc.tensor.matmul(ps[:, SQ_BLK:2 * SQ_BLK],
                                     kp[pr][HD:P, sks], qp[pr][HD:P, sqs],
                                     start=True, stop=True)
                    ex = epool.tile([P, 2 * SQ_BLK], BF16, tag="e")
                    nc.scalar.activation(ex[:], ps[:], EXPF)
                    for X in range(2):
                        h = 2 * pr + X
                        nc.tensor.matmul(
                            po_t[X][0:VW, :],
                            vbuf[:, sk, h * VW:(h + 1) * VW],
                            ex[:, X * SQ_BLK:(X + 1) * SQ_BLK],
                            start=(sk == 0), stop=(sk == NSK - 1))
                    # deferred consumers from the previous pair / sq
                    if sk == 1 and urgent:
                        urgent.pop(0)()
                    elif sk >= 2 and sk % 2 == 0 and pending:
                        pending.pop(0)()
                rcr = spool.tile([1, 2 * SQ_BLK], F32R, tag="rcr",
                                 name=f"rcr{pr}_{sq}")
                urgent.append(make_normalize_a(sq, pr, po_t, rcr))
                pending.append(make_normalize_b(sq, pr, rcr))
            last = sq == NSQ - 1
            for m in range(E // P):
                # final sq's o-proj alternates pools + eviction engines so
                # the tail pipelines with no idle engine
                pending.append(make_oproj(sq, m, use_psc=(last and m % 2 == 0),
                                          scalar_evict=(last and m % 2 == 1)))
        for u in urgent:
            u()
        for p in pending:
            p()

    nc.compile()
    return nc


_NC = None


def _get_nc():
    global _NC
    if _NC is None:
        _NC = _build_program()
    return _NC


def _bf(a):
    return np.ascontiguousarray(a.astype(NPBF16))


def _make_in_maps(inputs):
    q = np.asarray(inputs["query"], np.float32)
    k = np.asarray(inputs["key"], np.float32)
    v = np.asarray(inputs["value"], np.float32)
    Wq = np.asarray(inputs["Wq"], np.float32)
    Wk = np.asarray(inputs["Wk"], np.float32)
    Wv = np.asarray(inputs["Wv"], np.float32)
    Wo = np.asarray(inputs["Wo"], np.float32)
    bq = np.asarray(inputs["bq"], np.float32)
    bk = np.asarray(inputs["bk"], np.float32)
    bv = np.asarray(inputs["bv"], np.float32)
    scale = np.float32(HD ** -0.5)

    in_maps = []
    for c in range(NCORES):
        b = c // CORES_PER_BATCH
        h0 = (c % CORES_PER_BATCH) * HEADS_PER_CORE
        hsl = slice(h0 * HD, (h0 + HEADS_PER_CORE) * HD)
        bvh = bv[hsl].reshape(HEADS_PER_CORE, HD)
        bvb = np.concatenate(
            [bvh, np.ones((HEADS_PER_CORE, 1), np.float32)], axis=1).ravel()
        xqT = q[:, b, :].T                                  # [E, T]
        xq_blocks = np.concatenate(
            [xqT[:, n * SQ_BLK:(n + 1) * SQ_BLK] for n in range(NSQ)], axis=0)
        in_maps.append({
            "xq": _bf(xq_blocks),
            "xk": _bf(k[:, b, :].T),
            "xv": _bf(v[:, b, :].T),
            "wqt": _bf((Wq[hsl, :] * scale).T),
            "wkt": _bf(Wk[hsl, :].T),
            "wvt": _bf(Wv[hsl, :].T),
            "wot": _bf(Wo[:, hsl].T),
            "bqk": np.ascontiguousarray(
                np.concatenate([bq[hsl] * scale, bk[hsl]]).astype(np.float32)),
            "bvb": np.ascontiguousarray(bvb.astype(np.float32)),
        })
    return in_maps


def run_sharded(inputs, trace=False):
    """Returns (full_output [S,B,E] f32, BassKernelResults)."""
    nc = _get_nc()
    in_maps = _make_in_maps(inputs)
    res = run_bass_kernel_spmd(nc, in_maps, core_ids=list(range(NCORES)),
                               trace=trace)
    bo = np.asarray(inputs["bo"], np.float32)
    final = np.zeros((S, B, E), np.float32)
    for c in range(NCORES):
        b = c // CORES_PER_BATCH
        final[:, b, :] += res.results[c]["outp"].astype(np.float32).T
    final += bo
    return final, res


def kernel(**inputs):
    out, _ = run_sharded(inputs, trace=False)
    return out
